# revision 18
# baseline (speedup 1.0000x reference)
"""Symmetry-plane loss on 8 Trainium2 NeuronCores (Bass/Tile).

Shapes (hardcoded per spec):
  point_cloud    [64, 32768, 3] f32
  auxiliary_data [64, 32768, 3] f32   (closest-point grid, G = 32^3 = 32768)
  voxel_data     [64, 32768, 1] f32   (occupancy)
  planes         [3, 64, 4]     f32

Sharding: pure data parallel over batch B=64 -> 8 cores x 8 batches.
Each core computes a partial scalar (sym-sum + 25*reg-partial); host sums
the 8 partials, adds the reg identity constant, divides by B.

Per-core layout: 8 batches <-> 8 GPSIMD groups (16 partitions each).
Point p of batch b lives at (partition 16*b + p//2048, column p%2048).

The voxel-table gather runs on GPSIMD InstIndirectCopy from SBUF-resident
tables. Table rows per group (partition 16b+q):
  q=0: (vox,ax) bf16-pairs cells [0,16384)    q=4:  same, cells [16384,32768)
  q=8: (ay,az) bf16-pairs cells [0,16384)     q=12: same, cells [16384,32768)
Index fed = g & 16383; lo/hi resolved afterward by a predicated select with
mask (g >= 16384). The gather's output AP is strided so each 1024-index
chunk lands grouped by source partition, making the cross-partition
relayout DMAs fully contiguous.
"""
import os
import sys
import numpy as np

for _p in ("/opt/trn_rl_repo", "/root/.axon_site/_ro/trn_rl_repo"):
    if os.path.isdir(_p) and _p not in sys.path:
        sys.path.append(_p)

B, N, RES = 64, 32768, 32
SH = 8          # batches per core
P = 128
PPB = 2048      # points per partition
CH = 64         # g-columns per gather chunk (=> 1024 indices, Q7 scratch cap)
NCHUNK = PPB // CH   # 32 chunks per plane
SEGW_CONST = 256  # segment width for the gather/distance phases
WREG = np.float32(25.0)
EPS = np.float32(1e-6)

_CACHE = {}


def _fv(ap):
    """Flatten a [P, 1, n] view to [P, n]."""
    return ap.rearrange("p one n -> p (one n)")


def _build():
    import concourse.bass as bass
    import concourse.bacc as bacc
    import concourse.mybir as mybir
    import concourse.tile as tile
    from contextlib import ExitStack

    f32 = mybir.dt.float32
    u32 = mybir.dt.uint32
    u16 = mybir.dt.uint16
    bf16 = mybir.dt.bfloat16
    Alu = mybir.AluOpType
    Act = mybir.ActivationFunctionType

    nc = bacc.Bacc("TRN2", target_bir_lowering=False, debug=False, num_devices=8)
    pc_d = nc.dram_tensor("pc", [SH, N, 3], f32, kind="ExternalInput")
    aux_d = nc.dram_tensor("aux", [SH, N, 3], f32, kind="ExternalInput")
    vox_d = nc.dram_tensor("vox", [SH, N], f32, kind="ExternalInput")
    plb_d = nc.dram_tensor("plb", [3, P, 4], f32, kind="ExternalInput")
    out_d = nc.dram_tensor("out", [1, 1], f32, kind="ExternalOutput")

    with tile.TileContext(nc) as tc, ExitStack() as ctx:
        big = ctx.enter_context(tc.tile_pool(name="big", bufs=1))

        # pct+acc are allocated below tbl on purpose: the IndirectCopy ucode's
        # 3-index read pattern strays up to one table-length below/2x above the
        # row base, so tbl needs >=32KB of valid SBUF beneath it.
        pct = big.tile([P, 6144], f32, tag="pct")
        acc = big.tile([P, PPB], f32, tag="acc")
        tbl = big.tile([P, 8192], u32, tag="tbl")
        nc.vector.memset(acc[:], 0.0)
        # rows 4..7 and 12..15 of each group are never used; zero them so the
        # gather (which reads all 16 partitions of a group) sees defined memory
        nc.vector.memset(tbl[:], 0)

        nc.sync.dma_start(
            out=pct[:], in_=pc_d.rearrange("b (j p) c -> (b j) (p c)", j=16))

        # -- per-plane constants (per-partition broadcast comes pre-replicated
        #    from the host: plb[i, 16b+j, :] = planes[i, shard_b, :])
        cpl, ln2t = [], []
        for i in range(3):
            c = big.tile([P, 4], f32, tag=f"cpl{i}")
            nc.sync.dma_start(
                out=c[:], in_=plb_d.ap()[i:i + 1].rearrange("one p c -> (one p) c"))
            cpl.append(c)
            ln = big.tile([P, 1], f32, tag=f"ln{i}")
            t0 = big.tile([P, 1], f32, tag=f"lntmp{i}")
            nc.vector.tensor_tensor(out=ln[:], in0=c[:, 0:1], in1=c[:, 0:1], op=Alu.mult)
            nc.vector.tensor_tensor(out=t0[:], in0=c[:, 1:2], in1=c[:, 1:2], op=Alu.mult)
            nc.vector.tensor_tensor(out=ln[:], in0=ln[:], in1=t0[:], op=Alu.add)
            nc.vector.tensor_tensor(out=t0[:], in0=c[:, 2:3], in1=c[:, 2:3], op=Alu.mult)
            nc.vector.tensor_tensor(out=ln[:], in0=ln[:], in1=t0[:], op=Alu.add)
            nc.vector.reciprocal(out=ln[:], in_=ln[:])
            ln2t.append(ln)

        # -- build the gather tables --------------------------------------
        with tc.tile_pool(name="tbuild", bufs=1) as tbp:
            voxn = tbp.tile([P, PPB], f32, tag="voxn")
            nc.sync.dma_start(
                out=voxn[:], in_=vox_d.rearrange("b (j p) -> (b j) p", j=16))
            auxt = tbp.tile([P, 6144], f32, tag="auxt")
            nc.sync.dma_start(
                out=auxt[:], in_=aux_d.rearrange("b (j p) c -> (b j) (p c)", j=16))

            aux_r = auxt[:].rearrange("p (n c) -> p c n", c=3)
            pa = tbp.tile([P, PPB], u32, tag="pa")
            pb = tbp.tile([P, PPB], u32, tag="pb")
            pa_bf = pa[:].bitcast(bf16).rearrange("p (n t) -> p t n", t=2)
            pb_bf = pb[:].bitcast(bf16).rearrange("p (n t) -> p t n", t=2)
            nc.vector.tensor_copy(out=_fv(pa_bf[:, 0:1, :]), in_=voxn[:])
            nc.vector.tensor_copy(out=_fv(pa_bf[:, 1:2, :]), in_=_fv(aux_r[:, 0:1, :]))
            nc.vector.tensor_copy(out=_fv(pb_bf[:, 0:1, :]), in_=_fv(aux_r[:, 1:2, :]))
            nc.vector.tensor_copy(out=_fv(pb_bf[:, 1:2, :]), in_=_fv(aux_r[:, 2:3, :]))

            # row q of group b: q in 0..3 -> (vox,ax) pairs, cell range
            # [q*8192, (q+1)*8192); q in 8..11 -> (ay,az) pairs, range q-8.
            for srct, qbase in ((pa, 0), (pb, 8)):
                for r in range(4):
                    for b in range(SH):
                        nc.sync.dma_start(
                            out=tbl[16 * b + qbase + r:16 * b + qbase + r + 1, :],
                            in_=srct[16 * b + 4 * r:16 * b + 4 * r + 4, :])

        # -- per-plane: reflect -> voxel index -> packed u16 indices ------
        pg = ctx.enter_context(tc.tile_pool(name="gpipe", bufs=2))
        pw = ctx.enter_context(tc.tile_pool(name="gwork", bufs=1))

        pct_r = pct[:].rearrange("p (n c) -> p c n", c=3)
        xyz = [_fv(pct_r[:, c:c + 1, :]) for c in range(3)]

        gus, masks = [], []
        for i in range(3):
            c = cpl[i]
            nxs = [c[:, 0:1], c[:, 1:2], c[:, 2:3]]
            dd = c[:, 3:4]

            A = pw.tile([P, PPB], f32, tag="A")
            tmp = pw.tile([P, PPB], f32, tag="tmp")
            nc.vector.tensor_scalar(out=A[:], in0=xyz[0], scalar1=nxs[0],
                                    scalar2=None, op0=Alu.mult)
            nc.vector.tensor_scalar(out=tmp[:], in0=xyz[1], scalar1=nxs[1],
                                    scalar2=None, op0=Alu.mult)
            nc.vector.tensor_tensor(out=A[:], in0=A[:], in1=tmp[:], op=Alu.add)
            nc.vector.tensor_scalar(out=tmp[:], in0=xyz[2], scalar1=nxs[2],
                                    scalar2=None, op0=Alu.mult)
            nc.vector.tensor_tensor(out=A[:], in0=A[:], in1=tmp[:], op=Alu.add)
            # t = (p.n + d) / |n|^2 ; then A <- 2t
            nc.vector.tensor_scalar(out=A[:], in0=A[:], scalar1=dd,
                                    scalar2=ln2t[i][:], op0=Alu.add, op1=Alu.mult)
            nc.vector.tensor_scalar(out=A[:], in0=A[:], scalar1=2.0,
                                    scalar2=None, op0=Alu.mult)

            i32 = mybir.dt.int32
            gf = pw.tile([P, PPB], i32, tag="gf")
            uu = pw.tile([P, PPB], f32, tag="uu")
            uui = pw.tile([P, PPB], i32, tag="uui")
            for cc in range(3):
                # r_c = p_c - (2t)*n_c   (in place into tmp)
                nc.vector.tensor_scalar(out=tmp[:], in0=A[:], scalar1=nxs[cc],
                                        scalar2=None, op0=Alu.mult)
                nc.vector.tensor_tensor(out=tmp[:], in0=xyz[cc], in1=tmp[:],
                                        op=Alu.subtract)
                # u = trunc(clamp((r + 0.5) * 32, 0, 31))
                nc.scalar.activation(out=uu[:], in_=tmp[:], func=Act.Copy,
                                     bias=16.0, scale=32.0)
                nc.vector.tensor_scalar(out=uu[:], in0=uu[:], scalar1=0.0,
                                        scalar2=31.0, op0=Alu.max, op1=Alu.min)
                if cc == 0:
                    nc.vector.tensor_copy(out=gf[:], in_=uu[:])
                    nc.vector.tensor_scalar(out=gf[:], in0=gf[:], scalar1=1024,
                                            scalar2=None, op0=Alu.mult)
                else:
                    nc.vector.tensor_copy(out=uui[:], in_=uu[:])
                    if cc == 1:
                        nc.vector.tensor_scalar(out=uui[:], in0=uui[:],
                                                scalar1=32, scalar2=None,
                                                op0=Alu.mult)
                    nc.vector.tensor_tensor(out=gf[:], in0=gf[:], in1=uui[:],
                                            op=Alu.add)

            ms0 = pg.tile([P, PPB], mybir.dt.uint8, tag="mask0")
            ms1 = pg.tile([P, PPB], mybir.dt.uint8, tag="mask1")
            nc.vector.tensor_scalar(out=uui[:], in0=gf[:], scalar1=13,
                                    scalar2=1, op0=Alu.arith_shift_right,
                                    op1=Alu.bitwise_and)
            nc.vector.tensor_copy(out=ms0[:], in_=uui[:])
            nc.vector.tensor_scalar(out=ms1[:], in0=gf[:], scalar1=16384,
                                    scalar2=None, op0=Alu.is_ge)
            nc.vector.tensor_scalar(out=gf[:], in0=gf[:], scalar1=8191,
                                    scalar2=None, op0=Alu.bitwise_and)
            gu = pg.tile([P, PPB], u16, tag="gu")
            nc.vector.tensor_copy(out=gu[:], in_=gf[:])
            gus.append(gu); masks.append((ms0, ms1))

        # -- gather + select + distance -----------------------------------
        # Segment = 256 point-columns (4 gather chunks of 1024 indices).
        SEGW = 256
        CPS = SEGW // CH           # 4 chunks per segment
        NSEG = PPB // SEGW         # 8 segments per plane
        pgo = ctx.enter_context(tc.tile_pool(name="go", bufs=2))
        pT = ctx.enter_context(tc.tile_pool(name="T", bufs=2))
        psel = ctx.enter_context(tc.tile_pool(name="sel", bufs=2))
        pd = ctx.enter_context(tc.tile_pool(name="dist", bufs=1))

        _lvl = int(os.environ.get("BK_LEVEL", "4"))
        for i in range(3):
            gu, (ms0, ms1) = gus[i], masks[i]
            c = cpl[i]
            nxs = [c[:, 0:1], c[:, 1:2], c[:, 2:3]]
            dd = c[:, 3:4]
            if _lvl < 1:
                continue
            for seg in range(NSEG):
                lo = seg * SEGW
                # 4 gather chunks into the staging buffer; row 16b+k holds
                # table_k values for the group's points at (q, lo + cd),
                # stored at free offset q*SEGW + cd.
                go = pgo.tile([P, 16 * SEGW], u32, tag="go")
                go_q = go[:].rearrange("p (q cd) -> p q cd", q=16)
                for cc in range(CPS):
                    idx = gu[:, lo + cc * CH:lo + (cc + 1) * CH]
                    vw = go_q[:, :, cc * CH:(cc + 1) * CH].rearrange(
                        "p q d -> p d q")
                    if os.environ.get("BK_NO_GATHER"):
                        nc.vector.memset(go_q[:, :, cc * CH:(cc + 1) * CH], 0)
                    else:
                        nc.gpsimd.add_instruction(mybir.InstIndirectCopy(
                            name=f"I-{nc.next_id()}",
                            ins=[nc.gpsimd.lower_ap(tbl[:]),
                                 nc.gpsimd.lower_ap(idx)],
                            outs=[nc.gpsimd.lower_ap(vw)],
                            num_valid_indices=1024,
                        ))
                if _lvl < 2:
                    continue
                # relayout: one DMA per (group, useful table row)
                Ts = []
                for ty, k in enumerate((0, 1, 2, 3, 8, 9, 10, 11)):
                    T = pT.tile([P, SEGW], u32, tag=f"T{ty}")
                    for b in range(SH):
                        r0 = 16 * b + k
                        nc.sync.dma_start(out=T[16 * b:16 * (b + 1), :],
                                          in_=go[r0:r0 + 1, :])
                    Ts.append(T)
                if _lvl < 3:
                    continue
                m0 = ms0[:, lo:lo + SEGW]
                m1 = ms1[:, lo:lo + SEGW]
                selA = psel.tile([P, SEGW], u32, tag="selA")
                selB = psel.tile([P, SEGW], u32, tag="selB")
                s0 = psel.tile([P, SEGW], u32, tag="s0")
                s1 = psel.tile([P, SEGW], u32, tag="s1")
                for pair, out_t in ((0, selA), (1, selB)):
                    tb = Ts[4 * pair:4 * pair + 4]
                    nc.vector.select(out=s0[:], mask=m0,
                                     on_true=tb[1][:], on_false=tb[0][:])
                    nc.vector.select(out=s1[:], mask=m0,
                                     on_true=tb[3][:], on_false=tb[2][:])
                    nc.vector.select(out=out_t[:], mask=m1,
                                     on_true=s1[:], on_false=s0[:])

                if _lvl < 4:
                    continue
                if True:
                    # distance phase for this segment
                    sA = selA[:].bitcast(bf16).rearrange("p (n t) -> p t n", t=2)
                    sB = selB[:].bitcast(bf16).rearrange("p (n t) -> p t n", t=2)
                    vox_v = _fv(sA[:, 0:1, :])
                    axv = [_fv(sA[:, 1:2, :]), _fv(sB[:, 0:1, :]),
                           _fv(sB[:, 1:2, :])]
                    # recompute 2t for this quarter
                    da = pd.tile([P, SEGW], f32, tag="da")
                    t2 = pd.tile([P, SEGW], f32, tag="t2")
                    nc.vector.tensor_scalar(out=da[:], in0=xyz[0][:, lo:lo + SEGW],
                                            scalar1=nxs[0], scalar2=None,
                                            op0=Alu.mult)
                    nc.vector.tensor_scalar(out=t2[:], in0=xyz[1][:, lo:lo + SEGW],
                                            scalar1=nxs[1], scalar2=None,
                                            op0=Alu.mult)
                    nc.vector.tensor_tensor(out=da[:], in0=da[:], in1=t2[:],
                                            op=Alu.add)
                    nc.vector.tensor_scalar(out=t2[:], in0=xyz[2][:, lo:lo + SEGW],
                                            scalar1=nxs[2], scalar2=None,
                                            op0=Alu.mult)
                    nc.vector.tensor_tensor(out=da[:], in0=da[:], in1=t2[:],
                                            op=Alu.add)
                    nc.vector.tensor_scalar(out=da[:], in0=da[:], scalar1=dd,
                                            scalar2=ln2t[i][:], op0=Alu.add,
                                            op1=Alu.mult)
                    nc.vector.tensor_scalar(out=da[:], in0=da[:], scalar1=2.0,
                                            scalar2=None, op0=Alu.mult)
                    rr = []
                    for ccc in range(3):
                        rc = pd.tile([P, SEGW], f32, tag=f"rd{ccc}")
                        nc.vector.tensor_scalar(out=rc[:], in0=da[:],
                                                scalar1=nxs[ccc], scalar2=None,
                                                op0=Alu.mult)
                        nc.vector.tensor_tensor(
                            out=rc[:], in0=xyz[ccc][:, lo:lo + SEGW], in1=rc[:],
                            op=Alu.subtract)
                        # e = (r - tgt) + eps
                        nc.vector.tensor_tensor(out=rc[:], in0=rc[:],
                                                in1=axv[ccc], op=Alu.subtract)
                        nc.vector.tensor_scalar(out=rc[:], in0=rc[:],
                                                scalar1=float(EPS),
                                                scalar2=None, op0=Alu.add)
                        rr.append(rc)
                    sq = pd.tile([P, SEGW], f32, tag="sq")
                    nc.vector.tensor_tensor(out=sq[:], in0=rr[0][:],
                                            in1=rr[0][:], op=Alu.mult)
                    nc.vector.tensor_tensor(out=t2[:], in0=rr[1][:],
                                            in1=rr[1][:], op=Alu.mult)
                    nc.vector.tensor_tensor(out=sq[:], in0=sq[:], in1=t2[:],
                                            op=Alu.add)
                    nc.vector.tensor_tensor(out=t2[:], in0=rr[2][:],
                                            in1=rr[2][:], op=Alu.mult)
                    nc.vector.tensor_tensor(out=sq[:], in0=sq[:], in1=t2[:],
                                            op=Alu.add)
                    nc.scalar.activation(out=t2[:], in_=sq[:], func=Act.Sqrt)
                    wv = pd.tile([P, SEGW], f32, tag="wv")
                    nc.vector.tensor_scalar(out=wv[:], in0=vox_v, scalar1=1.0,
                                            scalar2=None, op0=Alu.subtract)
                    nc.vector.tensor_tensor(out=wv[:], in0=t2[:], in1=wv[:],
                                            op=Alu.mult)
                    # acc -= dist*(vox-1)  ==  acc += dist*(1-vox)
                    nc.vector.tensor_tensor(out=acc[:, lo:lo + SEGW],
                                            in0=acc[:, lo:lo + SEGW],
                                            in1=wv[:], op=Alu.subtract)

        # -- regularization term, computed on partition 0 (tiny) ----------
        # layout: row [1, 96] holding pl[i, b, c4] at offset i*32 + b*4 + c
        with tc.tile_pool(name="reg", bufs=1) as rp:
            plr = rp.tile([1, 96], f32, tag="plr")
            nc.sync.dma_start(
                out=plr[:].rearrange("one (i b c) -> one i b c", i=3, b=8),
                in_=plb_d.rearrange("i (b j) c -> i b j c", j=16)[:, :, 0:1, :]
                .rearrange("i b one c -> one i b c"))
            plv = plr[:].rearrange("one (i b c) -> one i b c", i=3, b=8)

            def comp(v, c):  # [1, 24] view of component c over (i, b)
                return v[:, :, :, c:c + 1].rearrange("one i b c -> one (i b c)")

            nrm = rp.tile([1, 24], f32, tag="nrm")
            t24 = rp.tile([1, 24], f32, tag="t24")
            nc.vector.tensor_tensor(out=nrm[:], in0=comp(plv, 0),
                                    in1=comp(plv, 0), op=Alu.mult)
            nc.vector.tensor_tensor(out=t24[:], in0=comp(plv, 1),
                                    in1=comp(plv, 1), op=Alu.mult)
            nc.vector.tensor_tensor(out=nrm[:], in0=nrm[:], in1=t24[:], op=Alu.add)
            nc.vector.tensor_tensor(out=t24[:], in0=comp(plv, 2),
                                    in1=comp(plv, 2), op=Alu.mult)
            nc.vector.tensor_tensor(out=nrm[:], in0=nrm[:], in1=t24[:], op=Alu.add)
            nc.scalar.activation(out=nrm[:], in_=nrm[:], func=Act.Sqrt)
            nc.vector.tensor_scalar(out=nrm[:], in0=nrm[:], scalar1=1e-12,
                                    scalar2=None, op0=Alu.max)
            nc.vector.reciprocal(out=nrm[:], in_=nrm[:])
            # nh row [1, 72]: offset(i, b, c) = i*24 + b*3 + c
            nh = rp.tile([1, 72], f32, tag="nh")
            nh_v = nh[:].rearrange("one (i b c) -> one i b c", i=3, b=8)
            for c in range(3):
                nc.vector.tensor_tensor(out=comp(nh_v, c), in0=comp(plv, c),
                                        in1=nrm[:], op=Alu.mult)
            # diag row [1, 24]: nh[i, b, i] -- 3 strided copies (stride 3)
            diag = rp.tile([1, 24], f32, tag="diag")
            for i in range(3):
                nc.vector.tensor_copy(
                    out=diag[:, 8 * i:8 * (i + 1)],
                    in_=nh_v[:, i:i + 1, :, i:i + 1].rearrange(
                        "one i b c -> one (i c b)"))
            # prod[i, b, j] = nh[i, b, j] * nh[j, b, i]
            nhT_v = nh[:].rearrange("one (j b i) -> one i b j", j=3, b=8)
            prod = rp.tile([1, 72], f32, tag="prod")
            prod_v = prod[:].rearrange("one (i b j) -> one i b j", i=3, b=8)
            nc.vector.tensor_tensor(out=prod_v, in0=nh_v, in1=nhT_v, op=Alu.mult)
            nc.vector.tensor_tensor(out=prod[:], in0=prod[:], in1=prod[:],
                                    op=Alu.mult)
            s1 = rp.tile([1, 1], f32, tag="s1")
            nc.vector.reduce_sum(out=s1[:], in_=prod[:], axis=mybir.AxisListType.X)
            nc.vector.tensor_tensor(out=diag[:], in0=diag[:], in1=diag[:],
                                    op=Alu.mult)
            d2 = rp.tile([1, 1], f32, tag="d2")
            nc.vector.reduce_sum(out=d2[:], in_=diag[:], axis=mybir.AxisListType.X)
            nc.vector.tensor_scalar(out=d2[:], in0=d2[:], scalar1=2.0,
                                    scalar2=None, op0=Alu.mult)
            nc.vector.tensor_tensor(out=s1[:], in0=s1[:], in1=d2[:], op=Alu.subtract)
            nc.vector.tensor_scalar(out=s1[:], in0=s1[:], scalar1=float(WREG),
                                    scalar2=None, op0=Alu.mult)

            vsum = rp.tile([P, 1], f32, tag="vsum")
            nc.vector.reduce_sum(out=vsum[:], in_=acc[:], axis=mybir.AxisListType.X)
            nc.vector.tensor_tensor(out=vsum[0:1, :], in0=vsum[0:1, :],
                                    in1=s1[:], op=Alu.add)

            ones = rp.tile([P, 1], f32, tag="ones")
            nc.vector.memset(ones[:], 1.0)
            with tc.tile_pool(name="ps", bufs=1, space="PSUM") as pp:
                ps = pp.tile([1, 1], f32, tag="ps")
                nc.tensor.matmul(ps[:], lhsT=ones[:], rhs=vsum[:])
                res = rp.tile([1, 1], f32, tag="res")
                nc.vector.tensor_copy(out=res[:], in_=ps[:])
                nc.sync.dma_start(out=out_d.ap(), in_=res[:])

    nc.compile()
    return nc


def _get_nc():
    if "nc" not in _CACHE:
        _CACHE["nc"] = _build()
    return _CACHE["nc"]


def _shard_inputs(pc, aux, vox, pl):
    maps = []
    for s in range(8):
        lo, hi = s * SH, (s + 1) * SH
        pls = pl[:, lo:hi, :]                                   # [3, 8, 4]
        plb = np.repeat(pls[:, :, None, :], 16, axis=2).reshape(3, P, 4)
        maps.append({
            "pc": np.ascontiguousarray(pc[lo:hi]),
            "aux": np.ascontiguousarray(aux[lo:hi]),
            "vox": np.ascontiguousarray(vox[lo:hi].reshape(SH, N)),
            "plb": np.ascontiguousarray(plb),
        })
    return maps


def _kernel_bass(pc, aux, vox, pl):
    from concourse.bass_utils import run_bass_kernel_spmd
    nc = _get_nc()
    maps = _shard_inputs(pc, aux, vox, pl)
    res = run_bass_kernel_spmd(nc, maps, core_ids=list(range(8)))
    total = np.float64(0.0)
    for r in res.results:
        total += np.float64(r["out"][0, 0])
    # + WREG * (3 per batch from the identity term), / B
    return np.float32(total / np.float64(B) + 75.0)


def _kernel_numpy(pc, aux, vox, pl):
    bidx = np.arange(B)[:, None]
    vox2 = vox.reshape(B, N)
    total = np.float32(0.0)
    for i in range(3):
        n = pl[i, :, :3]; d = pl[i, :, 3]
        ln2 = np.sum(n * n, axis=1)
        t = (np.einsum('bnc,bc->bn', pc, n) + d[:, None]) / ln2[:, None]
        pts = pc - np.float32(2.0) * t[:, :, None] * n[:, None, :]
        idx = ((pts + np.float32(0.5)) * np.float32(RES)).astype(np.int32)
        np.clip(idx, 0, RES - 1, out=idx)
        g = idx[..., 0] * 1024 + idx[..., 1] * 32 + idx[..., 2]
        v = vox2[bidx, g]; tgt = aux[bidx, g]
        diff = pts - tgt + EPS
        dist = np.sqrt(np.sum(diff * diff, axis=-1))
        total += np.sum(dist * (np.float32(1.0) - v), dtype=np.float32)
    nvec = pl[:, :, :3]
    nrm = np.maximum(np.sqrt(np.sum(nvec * nvec, axis=-1, keepdims=True)),
                     np.float32(1e-12))
    nv = np.transpose(nvec / nrm, (1, 0, 2))
    Me = nv * np.swapaxes(nv, 1, 2) - np.eye(3, dtype=np.float32)
    reg = np.sum(Me * Me, dtype=np.float32)
    return np.float32(total / B + WREG * reg / B)


def kernel(point_cloud, auxiliary_data, voxel_data, planes):
    pc = np.asarray(point_cloud, dtype=np.float32)
    aux = np.asarray(auxiliary_data, dtype=np.float32)
    vox = np.asarray(voxel_data, dtype=np.float32)
    pl = np.asarray(planes, dtype=np.float32)
    if os.environ.get("BK_FORCE_NUMPY"):
        return _kernel_numpy(pc, aux, vox, pl)
    try:
        return _kernel_bass(pc, aux, vox, pl)
    except Exception:
        import traceback
        traceback.print_exc()
        return _kernel_numpy(pc, aux, vox, pl)


# revision 19
# speedup vs baseline: 1.0027x; 1.0027x over previous
"""Symmetry-plane loss on 8 Trainium2 NeuronCores (Bass/Tile).

Shapes (hardcoded per spec):
  point_cloud    [64, 32768, 3] f32
  auxiliary_data [64, 32768, 3] f32   (closest-point grid, G = 32^3 = 32768)
  voxel_data     [64, 32768, 1] f32   (occupancy)
  planes         [3, 64, 4]     f32

Sharding: pure data parallel over batch B=64 -> 8 cores x 8 batches.
Each core computes a partial scalar (sym-sum + 25*reg-partial); host sums
the 8 partials, adds the reg identity constant, divides by B.

Per-core layout: 8 batches <-> 8 GPSIMD groups (16 partitions each).
Point p of batch b lives at (partition 16*b + p//2048, column p%2048).

The voxel-table gather runs on GPSIMD InstIndirectCopy from SBUF-resident
tables. Table rows per group (partition 16b+q):
  q=0: (vox,ax) bf16-pairs cells [0,16384)    q=4:  same, cells [16384,32768)
  q=8: (ay,az) bf16-pairs cells [0,16384)     q=12: same, cells [16384,32768)
Index fed = g & 16383; lo/hi resolved afterward by a predicated select with
mask (g >= 16384). The gather's output AP is strided so each 1024-index
chunk lands grouped by source partition, making the cross-partition
relayout DMAs fully contiguous.
"""
import os
import sys
import numpy as np

for _p in ("/opt/trn_rl_repo", "/root/.axon_site/_ro/trn_rl_repo"):
    if os.path.isdir(_p) and _p not in sys.path:
        sys.path.append(_p)

B, N, RES = 64, 32768, 32
SH = 8          # batches per core
P = 128
PPB = 2048      # points per partition
CH = 64         # g-columns per gather chunk (=> 1024 indices, Q7 scratch cap)
NCHUNK = PPB // CH   # 32 chunks per plane
SEGW_CONST = 256  # segment width for the gather/distance phases
WREG = np.float32(25.0)
EPS = np.float32(1e-6)

_CACHE = {}


def _fv(ap):
    """Flatten a [P, 1, n] view to [P, n]."""
    return ap.rearrange("p one n -> p (one n)")


def _build():
    import concourse.bass as bass
    import concourse.bacc as bacc
    import concourse.mybir as mybir
    import concourse.tile as tile
    from contextlib import ExitStack

    f32 = mybir.dt.float32
    u32 = mybir.dt.uint32
    u16 = mybir.dt.uint16
    bf16 = mybir.dt.bfloat16
    Alu = mybir.AluOpType
    Act = mybir.ActivationFunctionType

    nc = bacc.Bacc("TRN2", target_bir_lowering=False, debug=False, num_devices=8)
    pc_d = nc.dram_tensor("pc", [SH, N, 3], f32, kind="ExternalInput")
    aux_d = nc.dram_tensor("aux", [SH, N, 3], f32, kind="ExternalInput")
    vox_d = nc.dram_tensor("vox", [SH, N], f32, kind="ExternalInput")
    plb_d = nc.dram_tensor("plb", [3, P, 4], f32, kind="ExternalInput")
    out_d = nc.dram_tensor("out", [1, 1], f32, kind="ExternalOutput")

    with tile.TileContext(nc) as tc, ExitStack() as ctx:
        big = ctx.enter_context(tc.tile_pool(name="big", bufs=1))

        # pct+acc are allocated below tbl on purpose: the IndirectCopy ucode's
        # 3-index read pattern strays up to one table-length below/2x above the
        # row base, so tbl needs >=32KB of valid SBUF beneath it.
        pct = big.tile([P, 6144], f32, tag="pct")
        acc = big.tile([P, PPB], f32, tag="acc")
        tbl = big.tile([P, 8192], u32, tag="tbl")
        nc.vector.memset(acc[:], 0.0)
        # rows 4..7 and 12..15 of each group are never used; zero them so the
        # gather (which reads all 16 partitions of a group) sees defined memory
        nc.vector.memset(tbl[:], 0)

        nc.sync.dma_start(
            out=pct[:], in_=pc_d.rearrange("b (j p) c -> (b j) (p c)", j=16))

        # -- per-plane constants (per-partition broadcast comes pre-replicated
        #    from the host: plb[i, 16b+j, :] = planes[i, shard_b, :])
        cpl, ln2t = [], []
        for i in range(3):
            c = big.tile([P, 4], f32, tag=f"cpl{i}")
            nc.sync.dma_start(
                out=c[:], in_=plb_d.ap()[i:i + 1].rearrange("one p c -> (one p) c"))
            cpl.append(c)
            ln = big.tile([P, 1], f32, tag=f"ln{i}")
            t0 = big.tile([P, 1], f32, tag=f"lntmp{i}")
            nc.vector.tensor_tensor(out=ln[:], in0=c[:, 0:1], in1=c[:, 0:1], op=Alu.mult)
            nc.vector.tensor_tensor(out=t0[:], in0=c[:, 1:2], in1=c[:, 1:2], op=Alu.mult)
            nc.vector.tensor_tensor(out=ln[:], in0=ln[:], in1=t0[:], op=Alu.add)
            nc.vector.tensor_tensor(out=t0[:], in0=c[:, 2:3], in1=c[:, 2:3], op=Alu.mult)
            nc.vector.tensor_tensor(out=ln[:], in0=ln[:], in1=t0[:], op=Alu.add)
            nc.vector.reciprocal(out=ln[:], in_=ln[:])
            ln2t.append(ln)

        # -- build the gather tables --------------------------------------
        with tc.tile_pool(name="tbuild", bufs=1) as tbp:
            voxn = tbp.tile([P, PPB], f32, tag="voxn")
            nc.sync.dma_start(
                out=voxn[:], in_=vox_d.rearrange("b (j p) -> (b j) p", j=16))
            auxt = tbp.tile([P, 6144], f32, tag="auxt")
            nc.sync.dma_start(
                out=auxt[:], in_=aux_d.rearrange("b (j p) c -> (b j) (p c)", j=16))

            aux_r = auxt[:].rearrange("p (n c) -> p c n", c=3)
            pa = tbp.tile([P, PPB], u32, tag="pa")
            pb = tbp.tile([P, PPB], u32, tag="pb")
            pa_bf = pa[:].bitcast(bf16).rearrange("p (n t) -> p t n", t=2)
            pb_bf = pb[:].bitcast(bf16).rearrange("p (n t) -> p t n", t=2)
            nc.vector.tensor_copy(out=_fv(pa_bf[:, 0:1, :]), in_=voxn[:])
            nc.vector.tensor_copy(out=_fv(pa_bf[:, 1:2, :]), in_=_fv(aux_r[:, 0:1, :]))
            nc.vector.tensor_copy(out=_fv(pb_bf[:, 0:1, :]), in_=_fv(aux_r[:, 1:2, :]))
            nc.vector.tensor_copy(out=_fv(pb_bf[:, 1:2, :]), in_=_fv(aux_r[:, 2:3, :]))

            # row q of group b: q in 0..3 -> (vox,ax) pairs, cell range
            # [q*8192, (q+1)*8192); q in 8..11 -> (ay,az) pairs, range q-8.
            for srct, qbase in ((pa, 0), (pb, 8)):
                for r in range(4):
                    for b in range(SH):
                        nc.sync.dma_start(
                            out=tbl[16 * b + qbase + r:16 * b + qbase + r + 1, :],
                            in_=srct[16 * b + 4 * r:16 * b + 4 * r + 4, :])

        # -- per-plane: reflect -> voxel index -> packed u16 indices ------
        pg = ctx.enter_context(tc.tile_pool(name="gpipe", bufs=2))
        pw = ctx.enter_context(tc.tile_pool(name="gwork", bufs=1))

        pct_r = pct[:].rearrange("p (n c) -> p c n", c=3)
        xyz = [_fv(pct_r[:, c:c + 1, :]) for c in range(3)]

        gus, masks = [], []
        for i in range(3):
            c = cpl[i]
            nxs = [c[:, 0:1], c[:, 1:2], c[:, 2:3]]
            dd = c[:, 3:4]

            A = pw.tile([P, PPB], f32, tag="A")
            tmp = pw.tile([P, PPB], f32, tag="tmp")
            nc.vector.tensor_scalar(out=A[:], in0=xyz[0], scalar1=nxs[0],
                                    scalar2=None, op0=Alu.mult)
            nc.vector.tensor_scalar(out=tmp[:], in0=xyz[1], scalar1=nxs[1],
                                    scalar2=None, op0=Alu.mult)
            nc.vector.tensor_tensor(out=A[:], in0=A[:], in1=tmp[:], op=Alu.add)
            nc.vector.tensor_scalar(out=tmp[:], in0=xyz[2], scalar1=nxs[2],
                                    scalar2=None, op0=Alu.mult)
            nc.vector.tensor_tensor(out=A[:], in0=A[:], in1=tmp[:], op=Alu.add)
            # t = (p.n + d) / |n|^2 ; then A <- 2t
            nc.vector.tensor_scalar(out=A[:], in0=A[:], scalar1=dd,
                                    scalar2=ln2t[i][:], op0=Alu.add, op1=Alu.mult)
            nc.vector.tensor_scalar(out=A[:], in0=A[:], scalar1=2.0,
                                    scalar2=None, op0=Alu.mult)

            i32 = mybir.dt.int32
            gf = pw.tile([P, PPB], i32, tag="gf")
            uu = pw.tile([P, PPB], f32, tag="uu")
            uui = pw.tile([P, PPB], i32, tag="uui")
            for cc in range(3):
                # r_c = p_c - (2t)*n_c   (in place into tmp)
                nc.vector.tensor_scalar(out=tmp[:], in0=A[:], scalar1=nxs[cc],
                                        scalar2=None, op0=Alu.mult)
                nc.vector.tensor_tensor(out=tmp[:], in0=xyz[cc], in1=tmp[:],
                                        op=Alu.subtract)
                # u = trunc(clamp((r + 0.5) * 32, 0, 31)), same op order as
                # the reference
                nc.vector.tensor_scalar(out=uu[:], in0=tmp[:], scalar1=0.5,
                                        scalar2=32.0, op0=Alu.add, op1=Alu.mult)
                nc.vector.tensor_scalar(out=uu[:], in0=uu[:], scalar1=0.0,
                                        scalar2=31.0, op0=Alu.max, op1=Alu.min)
                if cc == 0:
                    nc.vector.tensor_copy(out=gf[:], in_=uu[:])
                    nc.vector.tensor_scalar(out=gf[:], in0=gf[:], scalar1=1024,
                                            scalar2=None, op0=Alu.mult)
                else:
                    nc.vector.tensor_copy(out=uui[:], in_=uu[:])
                    if cc == 1:
                        nc.vector.tensor_scalar(out=uui[:], in0=uui[:],
                                                scalar1=32, scalar2=None,
                                                op0=Alu.mult)
                    nc.vector.tensor_tensor(out=gf[:], in0=gf[:], in1=uui[:],
                                            op=Alu.add)

            ms0 = pg.tile([P, PPB], mybir.dt.uint8, tag="mask0")
            ms1 = pg.tile([P, PPB], mybir.dt.uint8, tag="mask1")
            nc.vector.tensor_scalar(out=uui[:], in0=gf[:], scalar1=13,
                                    scalar2=1, op0=Alu.arith_shift_right,
                                    op1=Alu.bitwise_and)
            nc.vector.tensor_copy(out=ms0[:], in_=uui[:])
            nc.vector.tensor_scalar(out=ms1[:], in0=gf[:], scalar1=16384,
                                    scalar2=None, op0=Alu.is_ge)
            nc.vector.tensor_scalar(out=gf[:], in0=gf[:], scalar1=8191,
                                    scalar2=None, op0=Alu.bitwise_and)
            gu = pg.tile([P, PPB], u16, tag="gu")
            nc.vector.tensor_copy(out=gu[:], in_=gf[:])
            gus.append(gu); masks.append((ms0, ms1))

        # -- gather + select + distance -----------------------------------
        # Segment = 256 point-columns (4 gather chunks of 1024 indices).
        SEGW = 256
        CPS = SEGW // CH           # 4 chunks per segment
        NSEG = PPB // SEGW         # 8 segments per plane
        pgo = ctx.enter_context(tc.tile_pool(name="go", bufs=2))
        pT = ctx.enter_context(tc.tile_pool(name="T", bufs=2))
        psel = ctx.enter_context(tc.tile_pool(name="sel", bufs=2))
        pd = ctx.enter_context(tc.tile_pool(name="dist", bufs=1))

        _lvl = int(os.environ.get("BK_LEVEL", "4"))
        for i in range(3):
            gu, (ms0, ms1) = gus[i], masks[i]
            c = cpl[i]
            nxs = [c[:, 0:1], c[:, 1:2], c[:, 2:3]]
            dd = c[:, 3:4]
            if _lvl < 1:
                continue
            for seg in range(NSEG):
                lo = seg * SEGW
                # 4 gather chunks into the staging buffer; row 16b+k holds
                # table_k values for the group's points at (q, lo + cd),
                # stored at free offset q*SEGW + cd.
                go = pgo.tile([P, 16 * SEGW], u32, tag="go")
                go_q = go[:].rearrange("p (q cd) -> p q cd", q=16)
                for cc in range(CPS):
                    idx = gu[:, lo + cc * CH:lo + (cc + 1) * CH]
                    vw = go_q[:, :, cc * CH:(cc + 1) * CH].rearrange(
                        "p q d -> p d q")
                    if os.environ.get("BK_NO_GATHER"):
                        nc.vector.memset(go_q[:, :, cc * CH:(cc + 1) * CH], 0)
                    else:
                        nc.gpsimd.add_instruction(mybir.InstIndirectCopy(
                            name=f"I-{nc.next_id()}",
                            ins=[nc.gpsimd.lower_ap(tbl[:]),
                                 nc.gpsimd.lower_ap(idx)],
                            outs=[nc.gpsimd.lower_ap(vw)],
                            num_valid_indices=1024,
                        ))
                if _lvl < 2:
                    continue
                # relayout: one DMA per (group, useful table row)
                Ts = []
                for ty, k in enumerate((0, 1, 2, 3, 8, 9, 10, 11)):
                    T = pT.tile([P, SEGW], u32, tag=f"T{ty}")
                    eng = nc.sync if ty % 2 == 0 else nc.scalar
                    for b in range(SH):
                        r0 = 16 * b + k
                        eng.dma_start(out=T[16 * b:16 * (b + 1), :],
                                      in_=go[r0:r0 + 1, :])
                    Ts.append(T)
                if _lvl < 3:
                    continue
                m0 = ms0[:, lo:lo + SEGW]
                m1 = ms1[:, lo:lo + SEGW]
                selA = psel.tile([P, SEGW], u32, tag="selA")
                selB = psel.tile([P, SEGW], u32, tag="selB")
                s0 = psel.tile([P, SEGW], u32, tag="s0")
                s1 = psel.tile([P, SEGW], u32, tag="s1")
                for pair, out_t in ((0, selA), (1, selB)):
                    tb = Ts[4 * pair:4 * pair + 4]
                    nc.vector.select(out=s0[:], mask=m0,
                                     on_true=tb[1][:], on_false=tb[0][:])
                    nc.vector.select(out=s1[:], mask=m0,
                                     on_true=tb[3][:], on_false=tb[2][:])
                    nc.vector.select(out=out_t[:], mask=m1,
                                     on_true=s1[:], on_false=s0[:])

                if _lvl < 4:
                    continue
                if True:
                    # distance phase for this segment
                    sA = selA[:].bitcast(bf16).rearrange("p (n t) -> p t n", t=2)
                    sB = selB[:].bitcast(bf16).rearrange("p (n t) -> p t n", t=2)
                    vox_v = _fv(sA[:, 0:1, :])
                    axv = [_fv(sA[:, 1:2, :]), _fv(sB[:, 0:1, :]),
                           _fv(sB[:, 1:2, :])]
                    # recompute 2t for this quarter
                    da = pd.tile([P, SEGW], f32, tag="da")
                    t2 = pd.tile([P, SEGW], f32, tag="t2")
                    nc.vector.tensor_scalar(out=da[:], in0=xyz[0][:, lo:lo + SEGW],
                                            scalar1=nxs[0], scalar2=None,
                                            op0=Alu.mult)
                    nc.vector.tensor_scalar(out=t2[:], in0=xyz[1][:, lo:lo + SEGW],
                                            scalar1=nxs[1], scalar2=None,
                                            op0=Alu.mult)
                    nc.vector.tensor_tensor(out=da[:], in0=da[:], in1=t2[:],
                                            op=Alu.add)
                    nc.vector.tensor_scalar(out=t2[:], in0=xyz[2][:, lo:lo + SEGW],
                                            scalar1=nxs[2], scalar2=None,
                                            op0=Alu.mult)
                    nc.vector.tensor_tensor(out=da[:], in0=da[:], in1=t2[:],
                                            op=Alu.add)
                    nc.vector.tensor_scalar(out=da[:], in0=da[:], scalar1=dd,
                                            scalar2=ln2t[i][:], op0=Alu.add,
                                            op1=Alu.mult)
                    nc.vector.tensor_scalar(out=da[:], in0=da[:], scalar1=2.0,
                                            scalar2=None, op0=Alu.mult)
                    rr = []
                    for ccc in range(3):
                        rc = pd.tile([P, SEGW], f32, tag=f"rd{ccc}")
                        nc.vector.tensor_scalar(out=rc[:], in0=da[:],
                                                scalar1=nxs[ccc], scalar2=None,
                                                op0=Alu.mult)
                        nc.vector.tensor_tensor(
                            out=rc[:], in0=xyz[ccc][:, lo:lo + SEGW], in1=rc[:],
                            op=Alu.subtract)
                        # e = (r - tgt) + eps
                        nc.vector.tensor_tensor(out=rc[:], in0=rc[:],
                                                in1=axv[ccc], op=Alu.subtract)
                        nc.vector.tensor_scalar(out=rc[:], in0=rc[:],
                                                scalar1=float(EPS),
                                                scalar2=None, op0=Alu.add)
                        rr.append(rc)
                    sq = pd.tile([P, SEGW], f32, tag="sq")
                    nc.vector.tensor_tensor(out=sq[:], in0=rr[0][:],
                                            in1=rr[0][:], op=Alu.mult)
                    nc.vector.tensor_tensor(out=t2[:], in0=rr[1][:],
                                            in1=rr[1][:], op=Alu.mult)
                    nc.vector.tensor_tensor(out=sq[:], in0=sq[:], in1=t2[:],
                                            op=Alu.add)
                    nc.vector.tensor_tensor(out=t2[:], in0=rr[2][:],
                                            in1=rr[2][:], op=Alu.mult)
                    nc.vector.tensor_tensor(out=sq[:], in0=sq[:], in1=t2[:],
                                            op=Alu.add)
                    nc.scalar.activation(out=t2[:], in_=sq[:], func=Act.Sqrt)
                    wv = pd.tile([P, SEGW], f32, tag="wv")
                    nc.vector.tensor_scalar(out=wv[:], in0=vox_v, scalar1=1.0,
                                            scalar2=None, op0=Alu.subtract)
                    nc.vector.tensor_tensor(out=wv[:], in0=t2[:], in1=wv[:],
                                            op=Alu.mult)
                    # acc -= dist*(vox-1)  ==  acc += dist*(1-vox)
                    nc.vector.tensor_tensor(out=acc[:, lo:lo + SEGW],
                                            in0=acc[:, lo:lo + SEGW],
                                            in1=wv[:], op=Alu.subtract)

        # -- regularization term, computed on partition 0 (tiny) ----------
        # layout: row [1, 96] holding pl[i, b, c4] at offset i*32 + b*4 + c
        with tc.tile_pool(name="reg", bufs=1) as rp:
            plr = rp.tile([1, 96], f32, tag="plr")
            nc.sync.dma_start(
                out=plr[:].rearrange("one (i b c) -> one i b c", i=3, b=8),
                in_=plb_d.rearrange("i (b j) c -> i b j c", j=16)[:, :, 0:1, :]
                .rearrange("i b one c -> one i b c"))
            plv = plr[:].rearrange("one (i b c) -> one i b c", i=3, b=8)

            def comp(v, c):  # [1, 24] view of component c over (i, b)
                return v[:, :, :, c:c + 1].rearrange("one i b c -> one (i b c)")

            nrm = rp.tile([1, 24], f32, tag="nrm")
            t24 = rp.tile([1, 24], f32, tag="t24")
            nc.vector.tensor_tensor(out=nrm[:], in0=comp(plv, 0),
                                    in1=comp(plv, 0), op=Alu.mult)
            nc.vector.tensor_tensor(out=t24[:], in0=comp(plv, 1),
                                    in1=comp(plv, 1), op=Alu.mult)
            nc.vector.tensor_tensor(out=nrm[:], in0=nrm[:], in1=t24[:], op=Alu.add)
            nc.vector.tensor_tensor(out=t24[:], in0=comp(plv, 2),
                                    in1=comp(plv, 2), op=Alu.mult)
            nc.vector.tensor_tensor(out=nrm[:], in0=nrm[:], in1=t24[:], op=Alu.add)
            nc.scalar.activation(out=nrm[:], in_=nrm[:], func=Act.Sqrt)
            nc.vector.tensor_scalar(out=nrm[:], in0=nrm[:], scalar1=1e-12,
                                    scalar2=None, op0=Alu.max)
            nc.vector.reciprocal(out=nrm[:], in_=nrm[:])
            # nh row [1, 72]: offset(i, b, c) = i*24 + b*3 + c
            nh = rp.tile([1, 72], f32, tag="nh")
            nh_v = nh[:].rearrange("one (i b c) -> one i b c", i=3, b=8)
            for c in range(3):
                nc.vector.tensor_tensor(out=comp(nh_v, c), in0=comp(plv, c),
                                        in1=nrm[:], op=Alu.mult)
            # diag row [1, 24]: nh[i, b, i] -- 3 strided copies (stride 3)
            diag = rp.tile([1, 24], f32, tag="diag")
            for i in range(3):
                nc.vector.tensor_copy(
                    out=diag[:, 8 * i:8 * (i + 1)],
                    in_=nh_v[:, i:i + 1, :, i:i + 1].rearrange(
                        "one i b c -> one (i c b)"))
            # prod[i, b, j] = nh[i, b, j] * nh[j, b, i]
            nhT_v = nh[:].rearrange("one (j b i) -> one i b j", j=3, b=8)
            prod = rp.tile([1, 72], f32, tag="prod")
            prod_v = prod[:].rearrange("one (i b j) -> one i b j", i=3, b=8)
            nc.vector.tensor_tensor(out=prod_v, in0=nh_v, in1=nhT_v, op=Alu.mult)
            nc.vector.tensor_tensor(out=prod[:], in0=prod[:], in1=prod[:],
                                    op=Alu.mult)
            s1 = rp.tile([1, 1], f32, tag="s1")
            nc.vector.reduce_sum(out=s1[:], in_=prod[:], axis=mybir.AxisListType.X)
            nc.vector.tensor_tensor(out=diag[:], in0=diag[:], in1=diag[:],
                                    op=Alu.mult)
            d2 = rp.tile([1, 1], f32, tag="d2")
            nc.vector.reduce_sum(out=d2[:], in_=diag[:], axis=mybir.AxisListType.X)
            nc.vector.tensor_scalar(out=d2[:], in0=d2[:], scalar1=2.0,
                                    scalar2=None, op0=Alu.mult)
            nc.vector.tensor_tensor(out=s1[:], in0=s1[:], in1=d2[:], op=Alu.subtract)
            nc.vector.tensor_scalar(out=s1[:], in0=s1[:], scalar1=float(WREG),
                                    scalar2=None, op0=Alu.mult)

            vsum = rp.tile([P, 1], f32, tag="vsum")
            nc.vector.reduce_sum(out=vsum[:], in_=acc[:], axis=mybir.AxisListType.X)
            nc.vector.tensor_tensor(out=vsum[0:1, :], in0=vsum[0:1, :],
                                    in1=s1[:], op=Alu.add)

            ones = rp.tile([P, 1], f32, tag="ones")
            nc.vector.memset(ones[:], 1.0)
            with tc.tile_pool(name="ps", bufs=1, space="PSUM") as pp:
                ps = pp.tile([1, 1], f32, tag="ps")
                nc.tensor.matmul(ps[:], lhsT=ones[:], rhs=vsum[:])
                res = rp.tile([1, 1], f32, tag="res")
                nc.vector.tensor_copy(out=res[:], in_=ps[:])
                nc.sync.dma_start(out=out_d.ap(), in_=res[:])

    nc.compile()
    return nc


def _get_nc():
    if "nc" not in _CACHE:
        _CACHE["nc"] = _build()
    return _CACHE["nc"]


def _shard_inputs(pc, aux, vox, pl):
    maps = []
    for s in range(8):
        lo, hi = s * SH, (s + 1) * SH
        pls = pl[:, lo:hi, :]                                   # [3, 8, 4]
        plb = np.repeat(pls[:, :, None, :], 16, axis=2).reshape(3, P, 4)
        maps.append({
            "pc": np.ascontiguousarray(pc[lo:hi]),
            "aux": np.ascontiguousarray(aux[lo:hi]),
            "vox": np.ascontiguousarray(vox[lo:hi].reshape(SH, N)),
            "plb": np.ascontiguousarray(plb),
        })
    return maps


def _kernel_bass(pc, aux, vox, pl):
    from concourse.bass_utils import run_bass_kernel_spmd
    nc = _get_nc()
    maps = _shard_inputs(pc, aux, vox, pl)
    res = run_bass_kernel_spmd(nc, maps, core_ids=list(range(8)))
    total = np.float64(0.0)
    for r in res.results:
        total += np.float64(r["out"][0, 0])
    # + WREG * (3 per batch from the identity term), / B
    return np.float32(total / np.float64(B) + 75.0)


def _kernel_numpy(pc, aux, vox, pl):
    bidx = np.arange(B)[:, None]
    vox2 = vox.reshape(B, N)
    total = np.float32(0.0)
    for i in range(3):
        n = pl[i, :, :3]; d = pl[i, :, 3]
        ln2 = np.sum(n * n, axis=1)
        t = (np.einsum('bnc,bc->bn', pc, n) + d[:, None]) / ln2[:, None]
        pts = pc - np.float32(2.0) * t[:, :, None] * n[:, None, :]
        idx = ((pts + np.float32(0.5)) * np.float32(RES)).astype(np.int32)
        np.clip(idx, 0, RES - 1, out=idx)
        g = idx[..., 0] * 1024 + idx[..., 1] * 32 + idx[..., 2]
        v = vox2[bidx, g]; tgt = aux[bidx, g]
        diff = pts - tgt + EPS
        dist = np.sqrt(np.sum(diff * diff, axis=-1))
        total += np.sum(dist * (np.float32(1.0) - v), dtype=np.float32)
    nvec = pl[:, :, :3]
    nrm = np.maximum(np.sqrt(np.sum(nvec * nvec, axis=-1, keepdims=True)),
                     np.float32(1e-12))
    nv = np.transpose(nvec / nrm, (1, 0, 2))
    Me = nv * np.swapaxes(nv, 1, 2) - np.eye(3, dtype=np.float32)
    reg = np.sum(Me * Me, dtype=np.float32)
    return np.float32(total / B + WREG * reg / B)


def kernel(point_cloud, auxiliary_data, voxel_data, planes):
    pc = np.asarray(point_cloud, dtype=np.float32)
    aux = np.asarray(auxiliary_data, dtype=np.float32)
    vox = np.asarray(voxel_data, dtype=np.float32)
    pl = np.asarray(planes, dtype=np.float32)
    if os.environ.get("BK_FORCE_NUMPY"):
        return _kernel_numpy(pc, aux, vox, pl)
    try:
        return _kernel_bass(pc, aux, vox, pl)
    except Exception:
        import traceback
        traceback.print_exc()
        return _kernel_numpy(pc, aux, vox, pl)
